# revision 33
# baseline (speedup 1.0000x reference)
"""Trainium2 Bass kernel for InterventionAwareStructure loss.

loss = sum_b,i,d A[b,i,d] * mask[regimes[b], d] / count   (scalar)

Data-parallel over batch across 8 NeuronCores. Each core:
  - streams its A shard [32, 512, 512] fp32 from HBM on the sync HWDGE
    ring at SDMA line rate (~27 GB/s/engine x 16): chunks 0-5 as whole
    4 MB transfers (32 KB per partition line), chunk 6 as 2 MB halves,
    and chunk 7 tapered down to two final 0.25 MB pieces so only ONE
    matmul + a [4, 512] store trail the last byte,
  - the otherwise-idle Vector engine folds each piece over the source
    axis with chains of contiguous pair-adds (ping-ponging between the
    piece tile and a scratch tile -- in-place adds miscompute on DVE),
    staying in full fp32 until the last level, whose f32r-typed
    destination is the rounding "producer" the BIR verifier requires;
    TensorE then needs only ~14 one-hot fp32r matmuls total, so
    neither compute engine ever paces the stream (a direct-matmul
    design sits at the fp32r PE roofline ~11 us/chunk > the 9.4 us
    DMA period, and couples into a metastable slow mode),
  - chunks 0-6 accumulate into PSUM bank A, whose 28 batch rows drain
    early through the scalar (ACT) engine -- ACT also issues the
    following out-DMA, so no cross-engine hop -- while the tail chunk
    accumulates into bank B rows 0-3 (its one-hot block is remapped so
    the [4, 512] PSUM read starts at partition 0),
  - the one-hot stationary table is synthesized on the idle GpSimd
    engine (memset + affine_select) while the stream warms up, so no
    weight bytes ride the HBM stream.

The mask gather (256x512), the mask dot, and the final scalar
reduction are all done on host; they are negligible next to the
256 MB stream of A.
"""

import numpy as np

import concourse.bass as bass
import concourse.tile as tile
from concourse import bacc, mybir
from concourse.bass_utils import run_bass_kernel_spmd

INTERVENTION_STRENGTH = 1.0

N_CORES = 8
B, N_REGIMES, D = 256, 16, 512
B_SH = B // N_CORES          # 32 batch items per core
NCHUNK = B_SH // 4           # 8 chunks of 4 batch items (4 MB fp32 each)
FREE = 4 * D * D // 128      # 8192 f32 per partition per chunk
HALF = FREE // 2
WCOLS = NCHUNK * 32

_CACHED_NC = None


def _build_nc() -> bass.Bass:
    nc = bacc.Bacc()
    f32 = mybir.dt.float32
    f32r = mybir.dt.float32r

    a = nc.dram_tensor("a", [B_SH, D, D], f32, kind="ExternalInput")
    out = nc.dram_tensor("out", [B_SH, D], f32, kind="ExternalOutput")

    # chunk g of batches (4g..4g+3) -> SBUF [128, FREE]: partition
    # p = (gb * 32 + ih) holds rows i = ih*16 + il of batch 4g+gb; free
    # axis = (il, d) with a contiguous 32 KB line per partition.
    a_view = a.rearrange(
        "(ng gb) (ih il) d -> ng (gb ih) (il d)", ng=NCHUNK, ih=32
    )
    # Same bytes tagged fp32r: the final tail pieces skip the DVE fold
    # and feed matmuls directly (a DMA producer passes the fp32r
    # verifier), so almost no work trails the final byte.
    ar_view = a.bitcast(f32r).rearrange(
        "(ng gb) (ih il) d -> ng (gb ih) (il d)", ng=NCHUNK, ih=32
    )

    mult = mybir.AluOpType.mult
    add = mybir.AluOpType.add

    with tile.TileContext(nc) as tc:
        with (
            tc.tile_pool(name="whole", bufs=3) as whole_pool,
            tc.tile_pool(name="part", bufs=3) as part_pool,
            tc.tile_pool(name="scratch", bufs=2) as scratch_pool,
            tc.tile_pool(name="hmm", bufs=4) as hmm_pool,
            tc.tile_pool(name="ptail", bufs=2) as ptail_pool,
            tc.tile_pool(name="small", bufs=1) as small_pool,
            tc.tile_pool(name="psum", bufs=2, space="PSUM") as psum_pool,
        ):
            # One-hot stationary table, built on the otherwise-idle
            # GpSimd engine while the stream warms up.  Blocks 0-6:
            # W[p, 32g + 4g + p//32] = 1 (chunk g -> PSUM rows 4g..).
            # Block 7 maps to rows 0-3 so the tail result is a
            # 32-partition-aligned [4, 512] PSUM read.
            w_f = small_pool.tile([128, WCOLS], f32)
            nc.gpsimd.memset(w_f[:], 1.0)
            for gb in range(4):
                nc.gpsimd.affine_select(
                    out=w_f[gb * 32:(gb + 1) * 32, :(NCHUNK - 1) * 32],
                    in_=w_f[gb * 32:(gb + 1) * 32, :(NCHUNK - 1) * 32],
                    pattern=[[-4, NCHUNK - 1], [1, 32]],
                    compare_op=mybir.AluOpType.is_equal,
                    fill=0.0,
                    base=-gb,
                    channel_multiplier=0,
                )
                nc.gpsimd.affine_select(
                    out=w_f[gb * 32:(gb + 1) * 32, (NCHUNK - 1) * 32:],
                    in_=w_f[gb * 32:(gb + 1) * 32, (NCHUNK - 1) * 32:],
                    pattern=[[1, 32]],
                    compare_op=mybir.AluOpType.is_equal,
                    fill=0.0,
                    base=-gb,
                    channel_multiplier=0,
                )
            # DVE copy re-tags the table fp32r for the matmul weights.
            w_t = small_pool.tile([128, WCOLS], f32r)
            nc.vector.tensor_copy(w_t[:], w_f[:])

            # Chunks 0-5: whole 4 MB transfers (best line rate).
            wtiles = []
            for g in range(6):
                a_t = whole_pool.tile([128, FREE], f32, tag="a")
                nc.sync.dma_start(a_t[:], a_view[g])
                wtiles.append(a_t)
            # Chunk 6 in 2 MB halves; chunk 7 tapered: 1 + 1 + 1 +
            # 0.5 MB fold pieces, then 0.25 + 0.25 MB raw-f32r pieces
            # feeding one matmul each.
            c6h = []
            for hb in (0, 1):
                h_t = part_pool.tile([128, HALF], f32, tag="p")
                nc.sync.dma_start(
                    h_t[:], a_view[6][:, hb * HALF:(hb + 1) * HALF]
                )
                c6h.append(h_t)
            t_a = part_pool.tile([128, HALF], f32, tag="p")
            nc.sync.dma_start(t_a[:, :2048], a_view[7][:, :2048])
            nc.sync.dma_start(t_a[:, 2048:], a_view[7][:, 2048:4096])
            t_b = part_pool.tile([128, HALF], f32, tag="p")
            nc.sync.dma_start(t_b[:, :2048], a_view[7][:, 4096:6144])
            nc.sync.dma_start(t_b[:, 2048:3072], a_view[7][:, 6144:7168])
            p1 = ptail_pool.tile([128, D], f32r)
            nc.sync.dma_start(p1[:], ar_view[7][:, 7168:7680])
            p2 = ptail_pool.tile([128, D], f32r)
            nc.sync.dma_start(p2[:], ar_view[7][:, 7680:8192])

            def pair_add(out_ap, in0_ap, in1_ap):
                nc.vector.scalar_tensor_tensor(
                    out=out_ap, in0=in0_ap, scalar=1.0, in1=in1_ap,
                    op0=mult, op1=add,
                )

            def fold(a_t, base, ncols):
                """Fold a_t[:, base:base+ncols] (ncols = 2^k * D) down
                to a [128, D] f32r tile with contiguous pair-adds,
                ping-ponging between a_t and a scratch tile."""
                s_t = None
                cur, off, w_c = a_t, base, ncols
                while True:
                    half_w = w_c // 2
                    if half_w == D:
                        h_t = hmm_pool.tile([128, D], f32r, tag="h")
                        pair_add(h_t[:], cur[:, off:off + D],
                                 cur[:, off + D:off + 2 * D])
                        return h_t
                    if s_t is None:
                        s_t = scratch_pool.tile([128, HALF], f32, tag="s")
                    nxt, noff = (s_t, 0) if cur is a_t else (a_t, base)
                    pair_add(nxt[:, noff:noff + half_w],
                             cur[:, off:off + half_w],
                             cur[:, off + half_w:off + w_c])
                    cur, off, w_c = nxt, noff, half_w

            # Folds in stream order; TensorE sees one matmul per fold.
            h4s = [fold(a_t, 0, FREE) for a_t in wtiles]
            h4s.append(fold(c6h[0], 0, HALF))
            h4s.append(fold(c6h[1], 0, HALF))

            # Chunks 0-6 accumulate into bank A (rows 0-27): chunk g's
            # one-hot block zeroes all other rows, so a single
            # start/stop group covers all eight matmuls.
            ps_a = psum_pool.tile([B_SH, D], f32, tag="psa")
            for k, h_t in enumerate(h4s):
                g = min(k, 6)
                nc.tensor.matmul(
                    ps_a[:], w_t[:, g * 32:(g + 1) * 32], h_t[:],
                    start=(k == 0), stop=(k == len(h4s) - 1),
                )
            nbat = 4 * (NCHUNK - 1)
            o_a = small_pool.tile([nbat, D], f32)
            nc.scalar.copy(o_a[:], ps_a[:nbat, :])
            nc.scalar.dma_start(out[:nbat, :], o_a[:])

            # Tail chunk into bank B rows 0-3.
            ps_b = psum_pool.tile([B_SH, D], f32, tag="psb")
            w_g = w_t[:, (NCHUNK - 1) * 32:]
            mm_b = [
                fold(t_a, 0, 2048),
                fold(t_a, 2048, 2048),
                fold(t_b, 0, 2048),
                fold(t_b, 2048, 1024),
                p1,
                p2,
            ]
            for k, h_t in enumerate(mm_b):
                nc.tensor.matmul(
                    ps_b[:], w_g, h_t[:],
                    start=(k == 0), stop=(k == len(mm_b) - 1),
                )
            o_b = small_pool.tile([4, D], f32)
            nc.scalar.copy(o_b[:], ps_b[:4, :])
            nc.scalar.dma_start(out[nbat:, :], o_b[:])

    nc.finalize()
    return nc


def _get_nc() -> bass.Bass:
    global _CACHED_NC
    if _CACHED_NC is None:
        _CACHED_NC = _build_nc()
    return _CACHED_NC


def _run(a_shards, **run_kwargs):
    nc = _get_nc()
    in_maps = [{"a": np.ascontiguousarray(a_shards[c])} for c in range(N_CORES)]
    return run_bass_kernel_spmd(nc, in_maps, list(range(N_CORES)), **run_kwargs)


def kernel(A_per_env, intervention_mask, regimes, _run_kwargs=None):
    A_per_env = np.asarray(A_per_env, dtype=np.float32)
    intervention_mask = np.asarray(intervention_mask, dtype=np.float32)
    regs = np.asarray(regimes).astype(np.int64)

    n_regimes = intervention_mask.shape[0]
    valid = regs < n_regimes
    e = np.clip(regs, 0, n_regimes - 1)
    masks = intervention_mask[e] * valid[:, None].astype(np.float32)  # [B, D]

    a_shards = [A_per_env[c * B_SH:(c + 1) * B_SH] for c in range(N_CORES)]

    res = _run(a_shards, **(_run_kwargs or {}))
    num = np.float64(0.0)
    for c in range(N_CORES):
        colsums = res.results[c]["out"].astype(np.float64)        # [32, 512]
        num += (colsums * masks[c * B_SH:(c + 1) * B_SH]).sum()

    count = masks.astype(np.float64).sum()
    loss = num / count if count > 0 else num
    out = np.asarray(INTERVENTION_STRENGTH * loss, dtype=np.float32)
    if _run_kwargs is not None:
        return out, res
    return out


# revision 35
# speedup vs baseline: 1.0078x; 1.0078x over previous
"""Trainium2 Bass kernel for InterventionAwareStructure loss.

loss = sum_b,i,d A[b,i,d] * mask[regimes[b], d] / count   (scalar)

Data-parallel over batch across 8 NeuronCores. Each core:
  - streams its A shard [32, 512, 512] fp32 from HBM on the sync HWDGE
    ring at SDMA line rate (~27 GB/s/engine x 16): chunks 0-5 as whole
    4 MB transfers (32 KB per partition line), chunk 6 as 2 MB halves,
    and chunk 7 tapered down to two final 0.25 MB pieces so only ONE
    matmul + a [4, 512] store trail the last byte,
  - the otherwise-idle Vector engine folds each piece over the source
    axis with chains of contiguous pair-adds (ping-ponging between the
    piece tile and a scratch tile -- in-place adds miscompute on DVE),
    staying in full fp32 until the last level, whose f32r-typed
    destination is the rounding "producer" the BIR verifier requires;
    TensorE then needs only ~14 one-hot fp32r matmuls total, so
    neither compute engine ever paces the stream (a direct-matmul
    design sits at the fp32r PE roofline ~11 us/chunk > the 9.4 us
    DMA period, and couples into a metastable slow mode),
  - chunks 0-6 accumulate into PSUM bank A, whose 28 batch rows drain
    early through the scalar (ACT) engine -- ACT also issues the
    following out-DMA, so no cross-engine hop -- while the tail chunk
    accumulates into bank B rows 0-3 (its one-hot block is remapped so
    the [4, 512] PSUM read starts at partition 0),
  - the one-hot stationary table is synthesized on the idle GpSimd
    engine (memset + affine_select) while the stream warms up, so no
    weight bytes ride the HBM stream.

The mask gather (256x512), the mask dot, and the final scalar
reduction are all done on host; they are negligible next to the
256 MB stream of A.
"""

import numpy as np

import concourse.bass as bass
import concourse.tile as tile
from concourse import bacc, mybir
from concourse.bass_utils import run_bass_kernel_spmd

INTERVENTION_STRENGTH = 1.0

N_CORES = 8
B, N_REGIMES, D = 256, 16, 512
B_SH = B // N_CORES          # 32 batch items per core
NCHUNK = B_SH // 4           # 8 chunks of 4 batch items (4 MB fp32 each)
FREE = 4 * D * D // 128      # 8192 f32 per partition per chunk
HALF = FREE // 2
WCOLS = NCHUNK * 32

_CACHED_NC = None


def _build_nc() -> bass.Bass:
    nc = bacc.Bacc()
    f32 = mybir.dt.float32
    f32r = mybir.dt.float32r

    a = nc.dram_tensor("a", [B_SH, D, D], f32, kind="ExternalInput")
    out = nc.dram_tensor("out", [B_SH, D], f32, kind="ExternalOutput")

    # chunk g of batches (4g..4g+3) -> SBUF [128, FREE]: partition
    # p = (gb * 32 + ih) holds rows i = ih*16 + il of batch 4g+gb; free
    # axis = (il, d) with a contiguous 32 KB line per partition.
    a_view = a.rearrange(
        "(ng gb) (ih il) d -> ng (gb ih) (il d)", ng=NCHUNK, ih=32
    )
    # Same bytes tagged fp32r: the final tail pieces skip the DVE fold
    # and feed matmuls directly (a DMA producer passes the fp32r
    # verifier), so almost no work trails the final byte.
    ar_view = a.bitcast(f32r).rearrange(
        "(ng gb) (ih il) d -> ng (gb ih) (il d)", ng=NCHUNK, ih=32
    )

    mult = mybir.AluOpType.mult
    add = mybir.AluOpType.add

    with tile.TileContext(nc) as tc:
        with (
            tc.tile_pool(name="part", bufs=10) as part_pool,
            tc.tile_pool(name="scratch", bufs=2) as scratch_pool,
            tc.tile_pool(name="hmm", bufs=4) as hmm_pool,
            tc.tile_pool(name="ptail", bufs=2) as ptail_pool,
            tc.tile_pool(name="small", bufs=1) as small_pool,
            tc.tile_pool(name="psum", bufs=2, space="PSUM") as psum_pool,
        ):
            # One-hot stationary table, built on the otherwise-idle
            # GpSimd engine while the stream warms up.  Blocks 0-6:
            # W[p, 32g + 4g + p//32] = 1 (chunk g -> PSUM rows 4g..).
            # Block 7 maps to rows 0-3 so the tail result is a
            # 32-partition-aligned [4, 512] PSUM read.
            w_f = small_pool.tile([128, WCOLS], f32)
            nc.gpsimd.memset(w_f[:], 1.0)
            for gb in range(4):
                nc.gpsimd.affine_select(
                    out=w_f[gb * 32:(gb + 1) * 32, :(NCHUNK - 1) * 32],
                    in_=w_f[gb * 32:(gb + 1) * 32, :(NCHUNK - 1) * 32],
                    pattern=[[-4, NCHUNK - 1], [1, 32]],
                    compare_op=mybir.AluOpType.is_equal,
                    fill=0.0,
                    base=-gb,
                    channel_multiplier=0,
                )
                nc.gpsimd.affine_select(
                    out=w_f[gb * 32:(gb + 1) * 32, (NCHUNK - 1) * 32:],
                    in_=w_f[gb * 32:(gb + 1) * 32, (NCHUNK - 1) * 32:],
                    pattern=[[1, 32]],
                    compare_op=mybir.AluOpType.is_equal,
                    fill=0.0,
                    base=-gb,
                    channel_multiplier=0,
                )
            # DVE copy re-tags the table fp32r for the matmul weights.
            w_t = small_pool.tile([128, WCOLS], f32r)
            nc.vector.tensor_copy(w_t[:], w_f[:])

            # Chunks 0-6 as 2 MB halves, one tile and one dma_start
            # each (16 KB lines).  With 10 bufs the pool recycles ~5
            # chunks behind the stream, so dma issue never couples to
            # the DVE folds (whole-chunk tiles with shallow bufs lock
            # into a metastable DVE-paced slow mode).
            htiles = []
            for g in range(NCHUNK - 1):
                for hb in (0, 1):
                    a_t = part_pool.tile([128, HALF], f32, tag="p")
                    nc.sync.dma_start(
                        a_t[:], a_view[g][:, hb * HALF:(hb + 1) * HALF]
                    )
                    htiles.append(a_t)
            # Chunk 7 tapered: 1 + 1 + 1 + 0.5 MB fold pieces, then
            # 0.25 + 0.25 MB raw-f32r pieces feeding one matmul each
            # (the first of them usually retires before the last byte).
            t_a = part_pool.tile([128, HALF], f32, tag="p")
            nc.sync.dma_start(t_a[:, :2048], a_view[7][:, :2048])
            nc.sync.dma_start(t_a[:, 2048:], a_view[7][:, 2048:4096])
            t_b = part_pool.tile([128, HALF], f32, tag="p")
            nc.sync.dma_start(t_b[:, :2048], a_view[7][:, 4096:6144])
            nc.sync.dma_start(t_b[:, 2048:3072], a_view[7][:, 6144:7168])
            p1 = ptail_pool.tile([128, D], f32r)
            nc.sync.dma_start(p1[:], ar_view[7][:, 7168:7680])
            p2 = ptail_pool.tile([128, D], f32r)
            nc.sync.dma_start(p2[:], ar_view[7][:, 7680:8192])

            def pair_add(out_ap, in0_ap, in1_ap):
                nc.vector.scalar_tensor_tensor(
                    out=out_ap, in0=in0_ap, scalar=1.0, in1=in1_ap,
                    op0=mult, op1=add,
                )

            def fold(a_t, base, ncols):
                """Fold a_t[:, base:base+ncols] (ncols = 2^k * D) down
                to a [128, D] f32r tile with contiguous pair-adds,
                ping-ponging between a_t and a scratch tile."""
                s_t = None
                cur, off, w_c = a_t, base, ncols
                while True:
                    half_w = w_c // 2
                    if half_w == D:
                        h_t = hmm_pool.tile([128, D], f32r, tag="h")
                        pair_add(h_t[:], cur[:, off:off + D],
                                 cur[:, off + D:off + 2 * D])
                        return h_t
                    if s_t is None:
                        s_t = scratch_pool.tile([128, HALF // 2], f32, tag="s")
                    nxt, noff = (s_t, 0) if cur is a_t else (a_t, base)
                    pair_add(nxt[:, noff:noff + half_w],
                             cur[:, off:off + half_w],
                             cur[:, off + half_w:off + w_c])
                    cur, off, w_c = nxt, noff, half_w

            # Folds in stream order; TensorE sees one matmul per fold.
            h4s = [fold(a_t, 0, HALF) for a_t in htiles]

            # Chunks 0-6 accumulate into bank A (rows 0-27): chunk g's
            # one-hot block zeroes all other rows, so a single
            # start/stop group covers all eight matmuls.
            ps_a = psum_pool.tile([B_SH, D], f32, tag="psa")
            for k, h_t in enumerate(h4s):
                g = k // 2
                nc.tensor.matmul(
                    ps_a[:], w_t[:, g * 32:(g + 1) * 32], h_t[:],
                    start=(k == 0), stop=(k == len(h4s) - 1),
                )
            nbat = 4 * (NCHUNK - 1)
            o_a = small_pool.tile([nbat, D], f32)
            nc.scalar.copy(o_a[:], ps_a[:nbat, :])
            nc.scalar.dma_start(out[:nbat, :], o_a[:])

            # Tail chunk into bank B rows 0-3.
            ps_b = psum_pool.tile([B_SH, D], f32, tag="psb")
            w_g = w_t[:, (NCHUNK - 1) * 32:]
            mm_b = [
                fold(t_a, 0, 2048),
                fold(t_a, 2048, 2048),
                fold(t_b, 0, 2048),
                fold(t_b, 2048, 1024),
                p1,
                p2,
            ]
            for k, h_t in enumerate(mm_b):
                nc.tensor.matmul(
                    ps_b[:], w_g, h_t[:],
                    start=(k == 0), stop=(k == len(mm_b) - 1),
                )
            o_b = small_pool.tile([4, D], f32)
            nc.scalar.copy(o_b[:], ps_b[:4, :])
            nc.scalar.dma_start(out[nbat:, :], o_b[:])

    nc.finalize()
    return nc


def _get_nc() -> bass.Bass:
    global _CACHED_NC
    if _CACHED_NC is None:
        _CACHED_NC = _build_nc()
    return _CACHED_NC


def _run(a_shards, **run_kwargs):
    nc = _get_nc()
    in_maps = [{"a": np.ascontiguousarray(a_shards[c])} for c in range(N_CORES)]
    return run_bass_kernel_spmd(nc, in_maps, list(range(N_CORES)), **run_kwargs)


def kernel(A_per_env, intervention_mask, regimes, _run_kwargs=None):
    A_per_env = np.asarray(A_per_env, dtype=np.float32)
    intervention_mask = np.asarray(intervention_mask, dtype=np.float32)
    regs = np.asarray(regimes).astype(np.int64)

    n_regimes = intervention_mask.shape[0]
    valid = regs < n_regimes
    e = np.clip(regs, 0, n_regimes - 1)
    masks = intervention_mask[e] * valid[:, None].astype(np.float32)  # [B, D]

    a_shards = [A_per_env[c * B_SH:(c + 1) * B_SH] for c in range(N_CORES)]

    res = _run(a_shards, **(_run_kwargs or {}))
    num = np.float64(0.0)
    for c in range(N_CORES):
        colsums = res.results[c]["out"].astype(np.float64)        # [32, 512]
        num += (colsums * masks[c * B_SH:(c + 1) * B_SH]).sum()

    count = masks.astype(np.float64).sum()
    loss = num / count if count > 0 else num
    out = np.asarray(INTERVENTION_STRENGTH * loss, dtype=np.float32)
    if _run_kwargs is not None:
        return out, res
    return out


# revision 36
# speedup vs baseline: 1.1875x; 1.1783x over previous
"""Trainium2 Bass kernel for InterventionAwareStructure loss.

loss = sum_b,i,d A[b,i,d] * mask[regimes[b], d] / count   (scalar)

Data-parallel over batch across 8 NeuronCores. Each core:
  - streams its A shard [32, 512, 512] fp32 from HBM on the sync HWDGE
    ring at SDMA line rate (~27 GB/s/engine x 16): chunks 0-6 as 2 MB
    halves (16 KB per partition line) through a 10-deep tile pool so
    DMA issue never couples to compute, and chunk 7 tapered down to
    two final 0.25 MB pieces so only ONE matmul + a [4, 512] store
    trail the last byte,
  - the otherwise-idle Vector engine folds each piece over the source
    axis with chains of contiguous pair-adds (ping-ponging between the
    piece tile and a scratch tile -- in-place adds miscompute on DVE),
    staying in full fp32 until the last level, whose f32r-typed
    destination is the rounding "producer" the BIR verifier requires;
    TensorE then needs only ~20 one-hot fp32r matmuls total, so
    neither compute engine ever paces the stream (a direct-matmul
    design sits at the fp32r PE roofline ~11 us/chunk > the 9.4 us
    DMA period, and couples into a metastable slow mode),
  - chunks 0-6 accumulate into PSUM bank A, whose 28 batch rows drain
    early through the scalar (ACT) engine -- ACT also issues the
    following out-DMA, so no cross-engine hop -- while the tail chunk
    accumulates into bank B rows 0-3 (its one-hot block is remapped so
    the [4, 512] PSUM read starts at partition 0),
  - the one-hot stationary table is synthesized on the idle GpSimd
    engine (memset + affine_select) while the stream warms up, so no
    weight bytes ride the HBM stream.

The mask gather (256x512), the mask dot, and the final scalar
reduction are all done on host; they are negligible next to the
256 MB stream of A.
"""

import numpy as np

import concourse.bass as bass
import concourse.tile as tile
from concourse import bacc, mybir
from concourse.bass_utils import run_bass_kernel_spmd

INTERVENTION_STRENGTH = 1.0

N_CORES = 8
B, N_REGIMES, D = 256, 16, 512
B_SH = B // N_CORES          # 32 batch items per core
NCHUNK = B_SH // 4           # 8 chunks of 4 batch items (4 MB fp32 each)
FREE = 4 * D * D // 128      # 8192 f32 per partition per chunk
HALF = FREE // 2
WCOLS = NCHUNK * 32

_CACHED_NC = None


def _build_nc() -> bass.Bass:
    nc = bacc.Bacc()
    f32 = mybir.dt.float32
    f32r = mybir.dt.float32r

    a = nc.dram_tensor("a", [B_SH, D, D], f32, kind="ExternalInput")
    out = nc.dram_tensor("out", [B_SH, D], f32, kind="ExternalOutput")

    # chunk g of batches (4g..4g+3) -> SBUF [128, FREE]: partition
    # p = (gb * 32 + ih) holds rows i = ih*16 + il of batch 4g+gb; free
    # axis = (il, d) with a contiguous 32 KB line per partition.
    a_view = a.rearrange(
        "(ng gb) (ih il) d -> ng (gb ih) (il d)", ng=NCHUNK, ih=32
    )
    # Same bytes tagged fp32r: the final tail pieces skip the DVE fold
    # and feed matmuls directly (a DMA producer passes the fp32r
    # verifier), so almost no work trails the final byte.
    ar_view = a.bitcast(f32r).rearrange(
        "(ng gb) (ih il) d -> ng (gb ih) (il d)", ng=NCHUNK, ih=32
    )

    mult = mybir.AluOpType.mult
    add = mybir.AluOpType.add

    with tile.TileContext(nc) as tc:
        with (
            tc.tile_pool(name="part", bufs=10) as part_pool,
            tc.tile_pool(name="scratch", bufs=2) as scratch_pool,
            tc.tile_pool(name="hmm", bufs=4) as hmm_pool,
            tc.tile_pool(name="ptail", bufs=2) as ptail_pool,
            tc.tile_pool(name="small", bufs=1) as small_pool,
            tc.tile_pool(name="psum", bufs=2, space="PSUM") as psum_pool,
        ):
            # One-hot stationary table, built on the otherwise-idle
            # GpSimd engine while the stream warms up.  Blocks 0-6:
            # W[p, 32g + 4g + p//32] = 1 (chunk g -> PSUM rows 4g..).
            # Block 7 maps to rows 0-3 so the tail result is a
            # 32-partition-aligned [4, 512] PSUM read.
            w_f = small_pool.tile([128, WCOLS], f32)
            nc.gpsimd.memset(w_f[:], 1.0)
            for gb in range(4):
                nc.gpsimd.affine_select(
                    out=w_f[gb * 32:(gb + 1) * 32, :(NCHUNK - 1) * 32],
                    in_=w_f[gb * 32:(gb + 1) * 32, :(NCHUNK - 1) * 32],
                    pattern=[[-4, NCHUNK - 1], [1, 32]],
                    compare_op=mybir.AluOpType.is_equal,
                    fill=0.0,
                    base=-gb,
                    channel_multiplier=0,
                )
                nc.gpsimd.affine_select(
                    out=w_f[gb * 32:(gb + 1) * 32, (NCHUNK - 1) * 32:],
                    in_=w_f[gb * 32:(gb + 1) * 32, (NCHUNK - 1) * 32:],
                    pattern=[[1, 32]],
                    compare_op=mybir.AluOpType.is_equal,
                    fill=0.0,
                    base=-gb,
                    channel_multiplier=0,
                )
            # DVE copy re-tags the table fp32r for the matmul weights.
            w_t = small_pool.tile([128, WCOLS], f32r)
            nc.vector.tensor_copy(w_t[:], w_f[:])

            # Chunks 0-6 as 2 MB halves, one tile and one dma_start
            # each (16 KB lines).  With 10 bufs the pool recycles ~5
            # chunks behind the stream, so dma issue never couples to
            # the DVE folds (whole-chunk tiles with shallow bufs lock
            # into a metastable DVE-paced slow mode).
            htiles = []
            for g in range(NCHUNK - 1):
                for hb in (0, 1):
                    a_t = part_pool.tile([128, HALF], f32, tag="p")
                    nc.sync.dma_start(
                        a_t[:], a_view[g][:, hb * HALF:(hb + 1) * HALF]
                    )
                    htiles.append(a_t)
            # Chunk 7 tapered: 1 + 1 + 1 + 0.5 MB fold pieces, then
            # 0.25 + 0.25 MB raw-f32r pieces feeding one matmul each
            # (the first of them usually retires before the last byte).
            t_a = part_pool.tile([128, HALF], f32, tag="p")
            nc.sync.dma_start(t_a[:, :2048], a_view[7][:, :2048])
            nc.sync.dma_start(t_a[:, 2048:], a_view[7][:, 2048:4096])
            t_b = part_pool.tile([128, HALF], f32, tag="p")
            nc.sync.dma_start(t_b[:, :2048], a_view[7][:, 4096:6144])
            nc.sync.dma_start(t_b[:, 2048:3072], a_view[7][:, 6144:7168])
            p1 = ptail_pool.tile([128, D], f32r)
            nc.sync.dma_start(p1[:], ar_view[7][:, 7168:7680])
            p2 = ptail_pool.tile([128, D], f32r)
            nc.sync.dma_start(p2[:], ar_view[7][:, 7680:8192])

            def pair_add(out_ap, in0_ap, in1_ap):
                nc.vector.scalar_tensor_tensor(
                    out=out_ap, in0=in0_ap, scalar=1.0, in1=in1_ap,
                    op0=mult, op1=add,
                )

            def fold(a_t, base, ncols):
                """Fold a_t[:, base:base+ncols] (ncols = 2^k * D) down
                to a [128, D] f32r tile with contiguous pair-adds,
                ping-ponging between a_t and a scratch tile."""
                s_t = None
                cur, off, w_c = a_t, base, ncols
                while True:
                    half_w = w_c // 2
                    if half_w == D:
                        h_t = hmm_pool.tile([128, D], f32r, tag="h")
                        pair_add(h_t[:], cur[:, off:off + D],
                                 cur[:, off + D:off + 2 * D])
                        return h_t
                    if s_t is None:
                        s_t = scratch_pool.tile([128, HALF // 2], f32, tag="s")
                    nxt, noff = (s_t, 0) if cur is a_t else (a_t, base)
                    pair_add(nxt[:, noff:noff + half_w],
                             cur[:, off:off + half_w],
                             cur[:, off + half_w:off + w_c])
                    cur, off, w_c = nxt, noff, half_w

            # Folds in stream order; TensorE sees one matmul per fold.
            h4s = [fold(a_t, 0, HALF) for a_t in htiles]

            # Chunks 0-6 accumulate into bank A (rows 0-27): chunk g's
            # one-hot block zeroes all other rows, so a single
            # start/stop group covers all eight matmuls.
            ps_a = psum_pool.tile([B_SH, D], f32, tag="psa")
            for k, h_t in enumerate(h4s):
                g = k // 2
                nc.tensor.matmul(
                    ps_a[:], w_t[:, g * 32:(g + 1) * 32], h_t[:],
                    start=(k == 0), stop=(k == len(h4s) - 1),
                )
            nbat = 4 * (NCHUNK - 1)
            o_a = small_pool.tile([nbat, D], f32)
            nc.scalar.copy(o_a[:], ps_a[:nbat, :])
            nc.scalar.dma_start(out[:nbat, :], o_a[:])

            # Tail chunk into bank B rows 0-3.
            ps_b = psum_pool.tile([B_SH, D], f32, tag="psb")
            w_g = w_t[:, (NCHUNK - 1) * 32:]
            mm_b = [
                fold(t_a, 0, 2048),
                fold(t_a, 2048, 2048),
                fold(t_b, 0, 2048),
                fold(t_b, 2048, 1024),
                p1,
                p2,
            ]
            for k, h_t in enumerate(mm_b):
                nc.tensor.matmul(
                    ps_b[:], w_g, h_t[:],
                    start=(k == 0), stop=(k == len(mm_b) - 1),
                )
            o_b = small_pool.tile([4, D], f32)
            nc.scalar.copy(o_b[:], ps_b[:4, :])
            nc.scalar.dma_start(out[nbat:, :], o_b[:])

    nc.finalize()
    return nc


def _get_nc() -> bass.Bass:
    global _CACHED_NC
    if _CACHED_NC is None:
        _CACHED_NC = _build_nc()
    return _CACHED_NC


def _run(a_shards, **run_kwargs):
    nc = _get_nc()
    in_maps = [{"a": np.ascontiguousarray(a_shards[c])} for c in range(N_CORES)]
    return run_bass_kernel_spmd(nc, in_maps, list(range(N_CORES)), **run_kwargs)


def kernel(A_per_env, intervention_mask, regimes, _run_kwargs=None):
    A_per_env = np.asarray(A_per_env, dtype=np.float32)
    intervention_mask = np.asarray(intervention_mask, dtype=np.float32)
    regs = np.asarray(regimes).astype(np.int64)

    n_regimes = intervention_mask.shape[0]
    valid = regs < n_regimes
    e = np.clip(regs, 0, n_regimes - 1)
    masks = intervention_mask[e] * valid[:, None].astype(np.float32)  # [B, D]

    a_shards = [A_per_env[c * B_SH:(c + 1) * B_SH] for c in range(N_CORES)]

    res = _run(a_shards, **(_run_kwargs or {}))
    num = np.float64(0.0)
    for c in range(N_CORES):
        colsums = res.results[c]["out"].astype(np.float64)        # [32, 512]
        num += (colsums * masks[c * B_SH:(c + 1) * B_SH]).sum()

    count = masks.astype(np.float64).sum()
    loss = num / count if count > 0 else num
    out = np.asarray(INTERVENTION_STRENGTH * loss, dtype=np.float32)
    if _run_kwargs is not None:
        return out, res
    return out
